# revision 36
# baseline (speedup 1.0000x reference)
"""Causal self-attention (B=1, S=4096, D=768, H=12) on 8 Trainium2 NeuronCores.

Sharding: sequence-parallel over queries with a stride-8 interleave
(core j owns queries j, j+8, j+16, ... -> perfectly causal-balanced AND
the SPMD program is identical on every core; per-core differences live
entirely in the input data: x slices and causal-mask tiles).

Per core (all tensors fp16 unless noted; fp16 matmuls run at the full
1 col/cycle PE rate and halve the HBM traffic of fp32 while keeping
element error ~0.05%, far below bf16's 0.4%):
  - projections q^T = Wq @ xq^T, k^T = Wk @ xkv^T, v = xkv @ Wv^T
    (host pre-transposes x slices and weights, pre-cast fp16).
  - k^T and v are AllGather'ed across the 8 cores (v gathered with a
    ones column per head so the softmax denominator falls out of the
    same PE matmul that computes A @ V).
  - attention in transposed-score layout S^T[kv, q]; scores stay ~N(0,1)
    so softmax needs no max-subtraction; exp on ACT straight out of PSUM.
  - local query chunk l (128 queries) needs kv chunks t in [0, 8l+8); with
    b = t//8 the score matmul covers query cols [128b:512] and exactly one
    [128,128] causal mask tile (shipped per-core from the host) applies at
    the leading 128 columns. kv chunks are processed in groups sharing one
    PSUM tile / one exp / one (strided) mask multiply.
  - y^T / l normalization via gpsimd partition-broadcast + DVE multiply.
  - fp16 output projection (row-parallel; no all-reduce needed).
"""

import sys

sys.path.insert(0, "/opt/trn_rl_repo")

import numpy as np
import ml_dtypes

import concourse.bass as bass
import concourse.mybir as mybir
import concourse.tile as tile
from concourse import bacc
from concourse.bass_utils import run_bass_kernel_spmd

NCORES = 8
S, D, H, HD = 4096, 768, 12, 64
P = 128
DMC = D // P            # 6 chunks of the model dim
NQ = S // NCORES        # 512 local queries per core
SLOT = S // NCORES      # 512 kv rows per core
HP = H // 2             # 6 head pairs
NKV = S // P            # 32 kv chunks of 128
VW = 65                 # v columns per head incl. ones column
F32 = mybir.dt.float32
FP16 = mybir.dt.float16
NP16 = np.float16
SCALE = 1.0 / np.sqrt(HD)

# kv-chunk groups per (head, head-pair): (b, [t...]) with b = t//8.
# Chunks in a group share one PSUM scores tile, one exp, one mask multiply.
GROUPS = [
    (0, [0, 1]), (0, [2, 3]), (0, [4, 5]), (0, [6, 7]),
    (1, [8, 9]), (1, [10, 11]), (1, [12, 13]), (1, [14, 15]),
    (2, [16, 17, 18, 19]), (2, [20, 21, 22, 23]),
    (3, list(range(24, 32))),
]
SLOTW = {0: 512, 1: 512, 2: 256, 3: 128}   # psum slot stride per b
CHN = {0: 512, 1: 384, 2: 256, 3: 128}     # matmul N per b

_CACHE = {}


def _build_program(reps: int = 1, no_cc: bool = False,
                   shadow_cc: bool = False):
    nc = bacc.Bacc("TRN2", target_bir_lowering=False, debug=False,
                   num_devices=NCORES)

    xqT = nc.dram_tensor("xqT", [D, NQ], FP16, kind="ExternalInput").ap()
    xkvT = nc.dram_tensor("xkvT", [D, SLOT], FP16, kind="ExternalInput").ap()
    wqT = nc.dram_tensor("wqT", [D, D], FP16, kind="ExternalInput").ap()
    wkT = nc.dram_tensor("wkT", [D, D], FP16, kind="ExternalInput").ap()
    wvT = nc.dram_tensor("wvT", [D, D], FP16, kind="ExternalInput").ap()
    wpT = nc.dram_tensor("wpT", [D, D], FP16, kind="ExternalInput").ap()
    masks = nc.dram_tensor("masks", [8, P, P], FP16, kind="ExternalInput").ap()
    out = nc.dram_tensor("out", [NQ, D], F32, kind="ExternalOutput").ap()
    ext_ag = None
    if no_cc:
        ext_ag = (
            nc.dram_tensor("kT_ag_in", [NCORES * D, SLOT], FP16,
                           kind="ExternalInput").ap(),
            nc.dram_tensor("v_ag_in", [S, H * VW], FP16,
                           kind="ExternalInput").ap(),
        )

    with tile.TileContext(nc, num_cores=NCORES) as tc:
        for _ in range(reps):
            _kernel_body(tc, xqT, xkvT, wqT, wkT, wvT, wpT, masks, out,
                         ext_ag=ext_ag, shadow_cc=shadow_cc)
    nc.compile()
    return nc


def _kernel_body(tc, xqT, xkvT, wqT, wkT, wvT, wpT, masks, out, ext_ag=None,
                 shadow_cc=False):
    nc = tc.nc
    rg = [list(range(NCORES))]

    with (
        tc.tile_pool(name="const", bufs=1) as cpool,
        tc.tile_pool(name="dram", bufs=1, space="DRAM") as dram,
    ):
        # ---- persistent SBUF tensors -------------------------------------
        xqT_sb = cpool.tile([P, DMC, NQ], FP16, tag="xqT")
        xkvT_sb = cpool.tile([P, DMC, SLOT], FP16, tag="xkvT")
        wqT_sb = cpool.tile([P, DMC, D], FP16, tag="wqT")
        wkT_sb = cpool.tile([P, DMC, D], FP16, tag="wkT")
        wvT_sb = cpool.tile([P, DMC, D], FP16, tag="wvT")
        wpT_sb = cpool.tile([P, DMC, D], FP16, tag="wpT")
        masks_sb = cpool.tile([P, 8, P], FP16, tag="masks")
        qT_sb = cpool.tile([P, DMC, NQ], FP16, tag="qT")
        kstage = cpool.tile([P, DMC, SLOT], FP16, tag="kstage")
        vstage = cpool.tile([P, SLOT // P, H, VW], FP16, tag="vstage")
        yT_sb = cpool.tile([P, DMC, NQ], FP16, tag="yT")
        # whole gathered V (+ones cols) resident in SBUF: [kv%128, chunk, col]
        v_sb = cpool.tile([P, NKV, H * VW], FP16, tag="v_sb")

        # ---- DRAM bounce + gathered buffers (each split in two so the
        # first half's AllGather fires while the second half computes) -----
        KB = D // 2
        kT_dram_a0 = dram.tile([P, SLOT], FP16)
        kT_dram_a1 = dram.tile([2 * P, SLOT], FP16)
        kT_dram_b = dram.tile([KB, SLOT], FP16)
        v_dram_a = dram.tile([SLOT, H * VW // 2], FP16)
        v_dram_b = dram.tile([SLOT, H * VW // 2], FP16)
        kT_ag_a0 = dram.tile([NCORES * P, SLOT], FP16, addr_space="Shared")
        kT_ag_a1 = dram.tile([NCORES * 2 * P, SLOT], FP16,
                             addr_space="Shared")
        kT_ag_b = dram.tile([NCORES * KB, SLOT], FP16, addr_space="Shared")
        v_ag_a = dram.tile([S, H * VW // 2], FP16, addr_space="Shared")
        v_ag_b = dram.tile([S, H * VW // 2], FP16, addr_space="Shared")

        # ---- load inputs (chunked so the first matmuls start early) ------
        xkvT_v = xkvT.rearrange("(c p) f -> p c f", p=P)
        wkT_v = wkT.rearrange("(c p) f -> p c f", p=P)
        for dmc in range(DMC):
            nc.sync.dma_start(out=wkT_sb[:, dmc, :], in_=wkT_v[:, dmc, :])
            nc.sync.dma_start(out=xkvT_sb[:, dmc, :], in_=xkvT_v[:, dmc, :])
        wvT_v = wvT.rearrange("(c p) f -> p c f", p=P)
        for dmc in range(DMC):
            nc.sync.dma_start(out=wvT_sb[:, dmc, :], in_=wvT_v[:, dmc, :])
        nc.sync.dma_start(out=xqT_sb, in_=xqT.rearrange("(c p) f -> p c f", p=P))
        nc.sync.dma_start(out=wqT_sb, in_=wqT.rearrange("(c p) f -> p c f", p=P))
        nc.sync.dma_start(out=wpT_sb, in_=wpT.rearrange("(c p) f -> p c f", p=P))
        nc.sync.dma_start(out=masks_sb, in_=masks.rearrange("u i q -> i u q"))
        nc.gpsimd.memset(vstage[:, :, :, 64:65], 1.0)

        # ---- K^T projection -> fp16 -> DRAM bounce; the AllGather for
        # head-pair 0 fires after the first row block (its scores are the
        # first gather consumer), head-pairs 1-2 follow ---------------------
        kT_dram_a1v = kT_dram_a1.rearrange("(c p) f -> p c f", p=P)
        kT_dram_bv = kT_dram_b.rearrange("(c p) f -> p c f", p=P)
        with tc.tile_pool(name="psum_k", bufs=2, space="PSUM") as pp:
            for oc in range(DMC):
                ps = pp.tile([P, SLOT], F32, tag="ps")
                for dmc in range(DMC):
                    nc.tensor.matmul(
                        ps,
                        wkT_sb[:, dmc, P * oc:P * (oc + 1)],
                        xkvT_sb[:, dmc, :],
                        start=(dmc == 0), stop=(dmc == DMC - 1),
                    )
                nc.vector.tensor_copy(kstage[:, oc, :], ps)
                if oc == 0:
                    nc.sync.dma_start(out=kT_dram_a0, in_=kstage[:, 0, :])
                elif oc < 3:
                    nc.sync.dma_start(out=kT_dram_a1v[:, oc - 1, :],
                                      in_=kstage[:, oc, :])
                else:
                    nc.sync.dma_start(out=kT_dram_bv[:, oc - 3, :],
                                      in_=kstage[:, oc, :])
                if (ext_ag is None or shadow_cc) and oc == 0:
                    nc.gpsimd.collective_compute(
                        "AllGather", mybir.AluOpType.bypass, replica_groups=rg,
                        ins=[kT_dram_a0.opt()], outs=[kT_ag_a0.opt()],
                    )
                if (ext_ag is None or shadow_cc) and oc == 2:
                    nc.gpsimd.collective_compute(
                        "AllGather", mybir.AluOpType.bypass, replica_groups=rg,
                        ins=[kT_dram_a1.opt()], outs=[kT_ag_a1.opt()],
                    )
            # hoisted first third of the Q projection (same pool: a pool
            # boundary here would act as a barrier); gives the attention its
            # first q chunk without waiting for the V projection
            for oc in range(2):
                ps = pp.tile([P, NQ], F32, tag="ps")
                for dmc in range(DMC):
                    nc.tensor.matmul(
                        ps,
                        wqT_sb[:, dmc, P * oc:P * (oc + 1)],
                        xqT_sb[:, dmc, :],
                        start=(dmc == 0), stop=(dmc == DMC - 1),
                    )
                nc.vector.tensor_copy(qT_sb[:, oc, :], ps)
        # ---- V projection -> fp16 (+ones col) -> DRAM bounce; heads 0-5
        # (og=0) complete and gather before heads 6-11 start ----------------
        with tc.tile_pool(name="psum_v", bufs=2, space="PSUM") as pp:
            for og in range(2):
                for sc in range(SLOT // P):
                    ps = pp.tile([P, 384], F32, tag="ps")
                    for dmc in range(DMC):
                        nc.tensor.matmul(
                            ps,
                            xkvT_sb[:, dmc, P * sc:P * (sc + 1)],
                            wvT_sb[:, dmc, 384 * og:384 * (og + 1)],
                            start=(dmc == 0), stop=(dmc == DMC - 1),
                        )
                    for hh in range(6):
                        h = 6 * og + hh
                        nc.vector.tensor_copy(
                            vstage[:, sc, h, 0:64], ps[:, 64 * hh:64 * (hh + 1)]
                        )
                vd = v_dram_a if og == 0 else v_dram_b
                nc.sync.dma_start(
                    out=vd.rearrange("(sc p) f -> p sc f", p=P),
                    in_=vstage[:, :, 6 * og:6 * (og + 1), :]
                        .rearrange("p sc h w -> p sc (h w)"),
                )
                if ext_ag is None or shadow_cc:
                    va, vo = ((v_dram_a, v_ag_a) if og == 0
                              else (v_dram_b, v_ag_b))
                    nc.gpsimd.collective_compute(
                        "AllGather", mybir.AluOpType.bypass, replica_groups=rg,
                        ins=[va.opt()], outs=[vo.opt()],
                    )
                    if og == 0:
                        # kb queued after va: va's data is consumed first
                        # (A@V of head-pair 0); kb isn't read until
                        # head-pair 3, so it must not delay va on the
                        # serial collective queue
                        nc.gpsimd.collective_compute(
                            "AllGather", mybir.AluOpType.bypass,
                            replica_groups=rg,
                            ins=[kT_dram_b.opt()], outs=[kT_ag_b.opt()],
                        )

        if ext_ag is not None:
            kT_ag_in, v_ag_in = ext_ag
            kT_in_r = kT_ag_in.rearrange("(s r) c -> r s c", r=D)
            kT_r_a0 = kT_in_r[0:P]
            kT_r_a1 = kT_in_r[P:3 * P]
            kT_r_b = kT_in_r[3 * P:D]
            v_a = v_ag_in[:, 0:H * VW // 2]
            v_b = v_ag_in[:, H * VW // 2:H * VW]
        else:
            kT_r_a0 = kT_ag_a0.rearrange("(s r) c -> r s c", r=P)
            kT_r_a1 = kT_ag_a1.rearrange("(s r) c -> r s c", r=2 * P)
            kT_r_b = kT_ag_b.rearrange("(s r) c -> r s c", r=KB)
            v_a, v_b = v_ag_a, v_ag_b
        HVW2 = H * VW // 2
        for si in range(4):
            rows = slice(1024 * si, 1024 * (si + 1))
            nc.sync.dma_start(
                out=v_sb[:, 8 * si:8 * (si + 1), 0:HVW2],
                in_=v_a[rows, :].rearrange("(t p) c -> p t c", p=P))
            nc.sync.dma_start(
                out=v_sb[:, 8 * si:8 * (si + 1), HVW2:2 * HVW2],
                in_=v_b[rows, :].rearrange("(t p) c -> p t c", p=P))

        # ---- Q^T projection -> fp16 (overlaps with the collective) -------
        with tc.tile_pool(name="psum_q", bufs=2, space="PSUM") as pp:
            for oc in range(2, DMC):
                ps = pp.tile([P, NQ], F32, tag="ps")
                for dmc in range(DMC):
                    nc.tensor.matmul(
                        ps,
                        wqT_sb[:, dmc, P * oc:P * (oc + 1)],
                        xqT_sb[:, dmc, :],
                        start=(dmc == 0), stop=(dmc == DMC - 1),
                    )
                nc.vector.tensor_copy(qT_sb[:, oc, :], ps)

        # ---- attention ----------------------------------------------------
        with (
            tc.tile_pool(name="kv", bufs=8) as kvpool,
            tc.tile_pool(name="att", bufs=16) as apool,
            tc.tile_pool(name="ps_s", bufs=3, space="PSUM") as spool,
            tc.tile_pool(name="ps_y", bufs=2, space="PSUM") as ypool,
            tc.tile_pool(name="norm", bufs=4) as npool,
        ):
            for hp in range(HP):
                ytiles = [ypool.tile([VW, NQ], F32, tag="y",
                                     name=f"y_{hp}_{hh}") for hh in range(2)]
                for gi, (b, ts) in enumerate(GROUPS):
                    C, SW, N = len(ts), SLOTW[b], CHN[b]
                    t0 = ts[0]
                    slot0, cb0 = t0 // 4, t0 % 4
                    # K chunk group: one DMA [128, C*128]
                    if hp == 0:
                        kT_r, kro = kT_r_a0, 0
                    elif hp < 3:
                        kT_r, kro = kT_r_a1, P * (hp - 1)
                    else:
                        kT_r, kro = kT_r_b, P * (hp - 3)
                    kbig = kvpool.tile([P, 8, P], FP16, tag="k",
                                       name=f"k_{hp}_{gi}")
                    if b < 3:
                        kin = kT_r[kro:kro + P, slot0, P * cb0:P * (cb0 + C)]
                    else:
                        kin = kT_r[kro:kro + P, 6:8, :]
                    nc.sync.dma_start(
                        out=kbig[:, 0:C, :].rearrange("p a b -> p (a b)"),
                        in_=kin)
                    for hh in range(2):
                        h = 2 * hp + hh
                        oc, ro = h // 2, 64 * (h % 2)
                        st = spool.tile([P, 1024], F32, tag="s",
                                        name=f"s_{hp}_{gi}_{hh}")
                        at = apool.tile([P, 1024], FP16, tag="a",
                                        name=f"a_{hp}_{gi}_{hh}")
                        for ci, t in enumerate(ts):
                            nc.tensor.matmul(
                                st[:, SW * ci:SW * ci + N],
                                kbig[64 * hh:64 * (hh + 1), ci, :],
                                qT_sb[ro:ro + 64, oc, P * b:NQ],
                                start=True, stop=True,
                            )
                        # exp over the packed group (gap-free via 3D AP)
                        if b == 1:
                            src = st.rearrange("p (g c) -> p g c", c=512)[:, :, 0:N]
                            dst = at.rearrange("p (g c) -> p g c", c=512)[:, :, 0:N]
                        else:
                            src = st[:, 0:C * SW]
                            dst = at[:, 0:C * SW]
                        nc.scalar.activation(
                            dst, src, mybir.ActivationFunctionType.Exp,
                            scale=float(SCALE),
                        )
                        # one strided mask multiply for the whole group
                        u0 = t0 % 8
                        av = at.rearrange("p (g c) -> p g c", c=SW)[:, 0:C, 0:P]
                        nc.vector.tensor_mul(av, av, masks_sb[:, u0:u0 + C, :])
                        # A @ [V | 1] accumulation; the leading
                        # 16(t-8b) at-columns are exact zeros from the
                        # causal mask, so the stream starts past them
                        for ci, t in enumerate(ts):
                            off = 16 * (t - 8 * b)
                            nc.tensor.matmul(
                                ytiles[hh][:, P * b + off:NQ],
                                v_sb[:, t, VW * h:VW * (h + 1)],
                                at[:, SW * ci + off:SW * ci + N],
                                start=(t == 0), stop=(t == NKV - 1),
                                skip_group_check=True,
                            )
                # normalize: y[0:64] * (1 / y[64]) -> yT_sb
                for hh in range(2):
                    h = 2 * hp + hh
                    oc, ro = h // 2, 64 * (h % 2)
                    r = npool.tile([1, NQ], F32, tag="r", name=f"r_{hp}_{hh}")
                    nc.vector.reciprocal(r, ytiles[hh][64:65, :])
                    rbs = npool.tile([64, NQ], F32, tag="rb",
                                     name=f"rb_{hp}_{hh}")
                    nc.gpsimd.partition_broadcast(rbs, r)
                    nc.vector.tensor_tensor(
                        out=yT_sb[ro:ro + 64, oc, :],
                        in0=ytiles[hh][0:64, :], in1=rbs,
                        op=mybir.AluOpType.mult,
                    )

        # ---- output projection (fp16) ------------------------------------
        with (
            tc.tile_pool(name="psum_o", bufs=2, space="PSUM") as pp,
            tc.tile_pool(name="ostage", bufs=3) as opool,
        ):
            for sc in range(NQ // P):
                for og in range(2):
                    ps = pp.tile([P, 384], F32, tag="ps")
                    for ic in range(DMC):
                        nc.tensor.matmul(
                            ps,
                            yT_sb[:, ic, P * sc:P * (sc + 1)],
                            wpT_sb[:, ic, 384 * og:384 * (og + 1)],
                            start=(ic == 0), stop=(ic == DMC - 1),
                        )
                    ost = opool.tile([P, 384], F32, tag="o")
                    nc.vector.tensor_copy(ost, ps)
                    nc.sync.dma_start(
                        out=out[P * sc:P * (sc + 1), 384 * og:384 * (og + 1)],
                        in_=ost,
                    )


def _host_masks(j: int) -> np.ndarray:
    u = np.arange(8)[:, None, None]
    i = np.arange(P)[None, :, None]
    p = np.arange(P)[None, None, :]
    m = (128 * u + i <= 8 * p + j)
    return m.astype(NP16)


def _make_in_maps(xf: np.ndarray, Wq, Wk, Wv, Wp) -> list[dict]:
    """xf: [S, D] fp32; W*: [D, D] fp32 (torch Linear convention y = x W^T)."""
    wts = {}
    for name, W in (("wqT", Wq), ("wkT", Wk), ("wvT", Wv), ("wpT", Wp)):
        wts[name] = np.ascontiguousarray(
            np.asarray(W, np.float32).T.astype(NP16))
    in_maps = []
    for j in range(NCORES):
        in_maps.append({
            "xqT": np.ascontiguousarray(xf[j::NCORES].T.astype(NP16)),
            "xkvT": np.ascontiguousarray(
                xf[SLOT * j:SLOT * (j + 1)].T.astype(NP16)),
            **wts, "masks": _host_masks(j),
        })
    return in_maps


def kernel(x, Wq, Wk, Wv, Wp, **_):
    x = np.asarray(x, dtype=np.float32)
    B = x.shape[0]
    xf = x.reshape(S, D)

    if "nc" not in _CACHE:
        _CACHE["nc"] = _build_program()
    nc = _CACHE["nc"]

    in_maps = _make_in_maps(xf, Wq, Wk, Wv, Wp)
    res = run_bass_kernel_spmd(nc, in_maps, list(range(NCORES)))
    out = np.empty((S, D), np.float32)
    for j in range(NCORES):
        out[j::NCORES] = res.results[j]["out"]
    return out.reshape(B, S, D)


if __name__ == "__main__":
    rng = np.random.default_rng(0)
    x = rng.standard_normal((1, S, D), dtype=np.float32)
    ws = [rng.standard_normal((D, D), dtype=np.float32) / np.sqrt(D)
          for _ in range(4)]
    y = kernel(x, *ws)
    print("ran", y.shape, y.dtype)


# revision 37
# speedup vs baseline: 1.0856x; 1.0856x over previous
"""Causal self-attention (B=1, S=4096, D=768, H=12) on 8 Trainium2 NeuronCores.

Sharding: sequence-parallel over queries with a stride-8 interleave
(core j owns queries j, j+8, j+16, ... -> perfectly causal-balanced AND
the SPMD program is identical on every core; per-core differences live
entirely in the input data: x slices and causal-mask tiles).

Per core (all tensors fp16 unless noted; fp16 matmuls run at the full
1 col/cycle PE rate and halve the HBM traffic of fp32 while keeping
element error ~0.05%, far below bf16's 0.4%):
  - projections q^T = Wq @ xq^T, k^T = Wk @ xkv^T, v = xkv @ Wv^T
    (host pre-transposes x slices and weights, pre-cast fp16).
  - k^T and v are AllGather'ed across the 8 cores (v gathered with a
    ones column per head so the softmax denominator falls out of the
    same PE matmul that computes A @ V).
  - attention in transposed-score layout S^T[kv, q]; scores stay ~N(0,1)
    so softmax needs no max-subtraction; exp on ACT straight out of PSUM.
  - local query chunk l (128 queries) needs kv chunks t in [0, 8l+8); with
    b = t//8 the score matmul covers query cols [128b:512] and exactly one
    [128,128] causal mask tile (shipped per-core from the host) applies at
    the leading 128 columns. kv chunks are processed in groups sharing one
    PSUM tile / one exp / one (strided) mask multiply.
  - y^T / l normalization via gpsimd partition-broadcast + DVE multiply.
  - fp16 output projection (row-parallel; no all-reduce needed).
"""

import sys

sys.path.insert(0, "/opt/trn_rl_repo")

import numpy as np
import ml_dtypes

import concourse.bass as bass
import concourse.mybir as mybir
import concourse.tile as tile
from concourse import bacc
from concourse.bass_utils import run_bass_kernel_spmd

NCORES = 8
S, D, H, HD = 4096, 768, 12, 64
P = 128
DMC = D // P            # 6 chunks of the model dim
NQ = S // NCORES        # 512 local queries per core
SLOT = S // NCORES      # 512 kv rows per core
HP = H // 2             # 6 head pairs
NKV = S // P            # 32 kv chunks of 128
VW = 65                 # v columns per head incl. ones column
F32 = mybir.dt.float32
FP16 = mybir.dt.float16
NP16 = np.float16
SCALE = 1.0 / np.sqrt(HD)

# kv-chunk groups per (head, head-pair): (b, [t...]) with b = t//8.
# Chunks in a group share one PSUM scores tile, one exp, one mask multiply.
GROUPS = [
    (0, [0, 1]), (0, [2, 3]), (0, [4, 5]), (0, [6, 7]),
    (1, [8, 9]), (1, [10, 11]), (1, [12, 13]), (1, [14, 15]),
    (2, [16, 17, 18, 19]), (2, [20, 21, 22, 23]),
    (3, list(range(24, 32))),
]
SLOTW = {0: 512, 1: 512, 2: 256, 3: 128}   # psum slot stride per b
CHN = {0: 512, 1: 384, 2: 256, 3: 128}     # matmul N per b

_CACHE = {}


def _build_program(reps: int = 1, no_cc: bool = False,
                   shadow_cc: bool = False):
    nc = bacc.Bacc("TRN2", target_bir_lowering=False, debug=False,
                   num_devices=NCORES)

    xqT = nc.dram_tensor("xqT", [D, NQ], FP16, kind="ExternalInput").ap()
    xkvT = nc.dram_tensor("xkvT", [D, SLOT], FP16, kind="ExternalInput").ap()
    wqT = nc.dram_tensor("wqT", [D, D], FP16, kind="ExternalInput").ap()
    wkT = nc.dram_tensor("wkT", [D, D], FP16, kind="ExternalInput").ap()
    wvT = nc.dram_tensor("wvT", [D, D], FP16, kind="ExternalInput").ap()
    wpT = nc.dram_tensor("wpT", [D, D], FP16, kind="ExternalInput").ap()
    masks = nc.dram_tensor("masks", [8, P, P], FP16, kind="ExternalInput").ap()
    out = nc.dram_tensor("out", [NQ, D], F32, kind="ExternalOutput").ap()
    ext_ag = None
    if no_cc:
        ext_ag = (
            nc.dram_tensor("kT_ag_in", [NCORES * D, SLOT], FP16,
                           kind="ExternalInput").ap(),
            nc.dram_tensor("v_ag_in", [S, H * VW], FP16,
                           kind="ExternalInput").ap(),
        )

    with tile.TileContext(nc, num_cores=NCORES) as tc:
        for _ in range(reps):
            _kernel_body(tc, xqT, xkvT, wqT, wkT, wvT, wpT, masks, out,
                         ext_ag=ext_ag, shadow_cc=shadow_cc)
    nc.compile()
    return nc


def _kernel_body(tc, xqT, xkvT, wqT, wkT, wvT, wpT, masks, out, ext_ag=None,
                 shadow_cc=False):
    nc = tc.nc
    rg = [list(range(NCORES))]

    with (
        tc.tile_pool(name="const", bufs=1) as cpool,
        tc.tile_pool(name="dram", bufs=1, space="DRAM") as dram,
    ):
        # ---- persistent SBUF tensors -------------------------------------
        xqT_sb = cpool.tile([P, DMC, NQ], FP16, tag="xqT")
        xkvT_sb = cpool.tile([P, DMC, SLOT], FP16, tag="xkvT")
        wqT_sb = cpool.tile([P, DMC, D], FP16, tag="wqT")
        wkT_sb = cpool.tile([P, DMC, D], FP16, tag="wkT")
        wvT_sb = cpool.tile([P, DMC, D], FP16, tag="wvT")
        wpT_sb = cpool.tile([P, DMC, D], FP16, tag="wpT")
        masks_sb = cpool.tile([P, 8, P], FP16, tag="masks")
        qT_sb = cpool.tile([P, DMC, NQ], FP16, tag="qT")
        kstage = cpool.tile([P, DMC, SLOT], FP16, tag="kstage")
        vstage = cpool.tile([P, SLOT // P, H, VW], FP16, tag="vstage")
        yT_sb = cpool.tile([P, DMC, NQ], FP16, tag="yT")
        # whole gathered V (+ones cols) resident in SBUF: [kv%128, chunk, col]
        v_sb = cpool.tile([P, NKV, H * VW], FP16, tag="v_sb")

        # ---- DRAM bounce + gathered buffers (each split in two so the
        # first half's AllGather fires while the second half computes) -----
        KB = D // 2
        kT_dram_a0 = dram.tile([P, SLOT], FP16)
        kT_dram_a1 = dram.tile([2 * P, SLOT], FP16)
        kT_dram_b = dram.tile([KB, SLOT], FP16)
        v_dram_a = dram.tile([SLOT, H * VW // 2], FP16)
        v_dram_b = dram.tile([SLOT, H * VW // 2], FP16)
        kT_ag_a0 = dram.tile([NCORES * P, SLOT], FP16, addr_space="Shared")
        kT_ag_a1 = dram.tile([NCORES * 2 * P, SLOT], FP16,
                             addr_space="Shared")
        kT_ag_b = dram.tile([NCORES * KB, SLOT], FP16, addr_space="Shared")
        v_ag_a = dram.tile([S, H * VW // 2], FP16, addr_space="Shared")
        v_ag_b = dram.tile([S, H * VW // 2], FP16, addr_space="Shared")

        # ---- load inputs (chunked so the first matmuls start early) ------
        xkvT_v = xkvT.rearrange("(c p) f -> p c f", p=P)
        wkT_v = wkT.rearrange("(c p) f -> p c f", p=P)
        for dmc in range(DMC):
            nc.sync.dma_start(out=wkT_sb[:, dmc, :], in_=wkT_v[:, dmc, :])
            nc.sync.dma_start(out=xkvT_sb[:, dmc, :], in_=xkvT_v[:, dmc, :])
        wvT_v = wvT.rearrange("(c p) f -> p c f", p=P)
        for dmc in range(DMC):
            nc.sync.dma_start(out=wvT_sb[:, dmc, :], in_=wvT_v[:, dmc, :])
        nc.sync.dma_start(out=xqT_sb, in_=xqT.rearrange("(c p) f -> p c f", p=P))
        nc.sync.dma_start(out=wqT_sb, in_=wqT.rearrange("(c p) f -> p c f", p=P))
        nc.sync.dma_start(out=wpT_sb, in_=wpT.rearrange("(c p) f -> p c f", p=P))
        nc.sync.dma_start(out=masks_sb, in_=masks.rearrange("u i q -> i u q"))
        nc.gpsimd.memset(vstage[:, :, :, 64:65], 1.0)

        # ---- K^T projection -> fp16 -> DRAM bounce; the AllGather for
        # head-pair 0 fires after the first row block (its scores are the
        # first gather consumer), head-pairs 1-2 follow ---------------------
        kT_dram_a1v = kT_dram_a1.rearrange("(c p) f -> p c f", p=P)
        kT_dram_bv = kT_dram_b.rearrange("(c p) f -> p c f", p=P)
        with tc.tile_pool(name="psum_k", bufs=2, space="PSUM") as pp:
            for oc in range(DMC):
                ps = pp.tile([P, SLOT], F32, tag="ps")
                for dmc in range(DMC):
                    nc.tensor.matmul(
                        ps,
                        wkT_sb[:, dmc, P * oc:P * (oc + 1)],
                        xkvT_sb[:, dmc, :],
                        start=(dmc == 0), stop=(dmc == DMC - 1),
                    )
                nc.vector.tensor_copy(kstage[:, oc, :], ps)
                if oc == 0:
                    nc.sync.dma_start(out=kT_dram_a0, in_=kstage[:, 0, :])
                elif oc < 3:
                    nc.sync.dma_start(out=kT_dram_a1v[:, oc - 1, :],
                                      in_=kstage[:, oc, :])
                else:
                    nc.sync.dma_start(out=kT_dram_bv[:, oc - 3, :],
                                      in_=kstage[:, oc, :])
                if (ext_ag is None or shadow_cc) and oc == 0:
                    nc.gpsimd.collective_compute(
                        "AllGather", mybir.AluOpType.bypass, replica_groups=rg,
                        ins=[kT_dram_a0.opt()], outs=[kT_ag_a0.opt()],
                    )
                if (ext_ag is None or shadow_cc) and oc == 2:
                    nc.gpsimd.collective_compute(
                        "AllGather", mybir.AluOpType.bypass, replica_groups=rg,
                        ins=[kT_dram_a1.opt()], outs=[kT_ag_a1.opt()],
                    )
            # hoisted first third of the Q projection (same pool: a pool
            # boundary here would act as a barrier); gives the attention its
            # first q chunk without waiting for the V projection
            for oc in range(2):
                ps = pp.tile([P, NQ], F32, tag="ps")
                for dmc in range(DMC):
                    nc.tensor.matmul(
                        ps,
                        wqT_sb[:, dmc, P * oc:P * (oc + 1)],
                        xqT_sb[:, dmc, :],
                        start=(dmc == 0), stop=(dmc == DMC - 1),
                    )
                nc.vector.tensor_copy(qT_sb[:, oc, :], ps)
        # ---- V projection -> fp16 (+ones col) -> DRAM bounce; heads 0-5
        # (og=0) complete and gather before heads 6-11 start ----------------
        with tc.tile_pool(name="psum_v", bufs=2, space="PSUM") as pp:
            for og in range(2):
                for sc in range(SLOT // P):
                    ps = pp.tile([P, 384], F32, tag="ps")
                    for dmc in range(DMC):
                        nc.tensor.matmul(
                            ps,
                            xkvT_sb[:, dmc, P * sc:P * (sc + 1)],
                            wvT_sb[:, dmc, 384 * og:384 * (og + 1)],
                            start=(dmc == 0), stop=(dmc == DMC - 1),
                        )
                    for hh in range(6):
                        h = 6 * og + hh
                        nc.vector.tensor_copy(
                            vstage[:, sc, h, 0:64], ps[:, 64 * hh:64 * (hh + 1)]
                        )
                vd = v_dram_a if og == 0 else v_dram_b
                nc.sync.dma_start(
                    out=vd.rearrange("(sc p) f -> p sc f", p=P),
                    in_=vstage[:, :, 6 * og:6 * (og + 1), :]
                        .rearrange("p sc h w -> p sc (h w)"),
                )
                if ext_ag is None or shadow_cc:
                    va, vo = ((v_dram_a, v_ag_a) if og == 0
                              else (v_dram_b, v_ag_b))
                    nc.gpsimd.collective_compute(
                        "AllGather", mybir.AluOpType.bypass, replica_groups=rg,
                        ins=[va.opt()], outs=[vo.opt()],
                    )
                    if og == 0:
                        # kb queued after va: va's data is consumed first
                        # (A@V of head-pair 0); kb isn't read until
                        # head-pair 3, so it must not delay va on the
                        # serial collective queue
                        nc.gpsimd.collective_compute(
                            "AllGather", mybir.AluOpType.bypass,
                            replica_groups=rg,
                            ins=[kT_dram_b.opt()], outs=[kT_ag_b.opt()],
                        )

        if ext_ag is not None:
            kT_ag_in, v_ag_in = ext_ag
            kT_in_r = kT_ag_in.rearrange("(s r) c -> r s c", r=D)
            kT_r_a0 = kT_in_r[0:P]
            kT_r_a1 = kT_in_r[P:3 * P]
            kT_r_b = kT_in_r[3 * P:D]
            v_a = v_ag_in[:, 0:H * VW // 2]
            v_b = v_ag_in[:, H * VW // 2:H * VW]
        else:
            kT_r_a0 = kT_ag_a0.rearrange("(s r) c -> r s c", r=P)
            kT_r_a1 = kT_ag_a1.rearrange("(s r) c -> r s c", r=2 * P)
            kT_r_b = kT_ag_b.rearrange("(s r) c -> r s c", r=KB)
            v_a, v_b = v_ag_a, v_ag_b
        HVW2 = H * VW // 2
        for si in range(4):
            rows = slice(1024 * si, 1024 * (si + 1))
            nc.sync.dma_start(
                out=v_sb[:, 8 * si:8 * (si + 1), 0:HVW2],
                in_=v_a[rows, :].rearrange("(t p) c -> p t c", p=P))
            nc.sync.dma_start(
                out=v_sb[:, 8 * si:8 * (si + 1), HVW2:2 * HVW2],
                in_=v_b[rows, :].rearrange("(t p) c -> p t c", p=P))

        # ---- Q^T projection -> fp16 (overlaps with the collective) -------
        with tc.tile_pool(name="psum_q", bufs=2, space="PSUM") as pp:
            for oc in range(2, DMC):
                ps = pp.tile([P, NQ], F32, tag="ps")
                for dmc in range(DMC):
                    nc.tensor.matmul(
                        ps,
                        wqT_sb[:, dmc, P * oc:P * (oc + 1)],
                        xqT_sb[:, dmc, :],
                        start=(dmc == 0), stop=(dmc == DMC - 1),
                    )
                nc.vector.tensor_copy(qT_sb[:, oc, :], ps)

        # ---- attention ----------------------------------------------------
        with (
            tc.tile_pool(name="kv", bufs=11) as kvpool,
            tc.tile_pool(name="att", bufs=22) as apool,
            tc.tile_pool(name="ps_s", bufs=3, space="PSUM") as spool,
            tc.tile_pool(name="ps_y", bufs=2, space="PSUM") as ypool,
            tc.tile_pool(name="norm", bufs=4) as npool,
        ):
            for hp in range(HP):
                ytiles = [ypool.tile([VW, NQ], F32, tag="y",
                                     name=f"y_{hp}_{hh}") for hh in range(2)]
                for gi, (b, ts) in enumerate(GROUPS):
                    C, SW, N = len(ts), SLOTW[b], CHN[b]
                    t0 = ts[0]
                    slot0, cb0 = t0 // 4, t0 % 4
                    # K chunk group: one DMA [128, C*128]
                    if hp == 0:
                        kT_r, kro = kT_r_a0, 0
                    elif hp < 3:
                        kT_r, kro = kT_r_a1, P * (hp - 1)
                    else:
                        kT_r, kro = kT_r_b, P * (hp - 3)
                    kbig = kvpool.tile([P, 8, P], FP16, tag="k",
                                       name=f"k_{hp}_{gi}")
                    if b < 3:
                        kin = kT_r[kro:kro + P, slot0, P * cb0:P * (cb0 + C)]
                    else:
                        kin = kT_r[kro:kro + P, 6:8, :]
                    nc.sync.dma_start(
                        out=kbig[:, 0:C, :].rearrange("p a b -> p (a b)"),
                        in_=kin)
                    for hh in range(2):
                        h = 2 * hp + hh
                        oc, ro = h // 2, 64 * (h % 2)
                        st = spool.tile([P, 1024], F32, tag="s",
                                        name=f"s_{hp}_{gi}_{hh}")
                        at = apool.tile([P, 1024], FP16, tag="a",
                                        name=f"a_{hp}_{gi}_{hh}")
                        for ci, t in enumerate(ts):
                            nc.tensor.matmul(
                                st[:, SW * ci:SW * ci + N],
                                kbig[64 * hh:64 * (hh + 1), ci, :],
                                qT_sb[ro:ro + 64, oc, P * b:NQ],
                                start=True, stop=True,
                            )
                        # exp over the packed group (gap-free via 3D AP)
                        if b == 1:
                            src = st.rearrange("p (g c) -> p g c", c=512)[:, :, 0:N]
                            dst = at.rearrange("p (g c) -> p g c", c=512)[:, :, 0:N]
                        else:
                            src = st[:, 0:C * SW]
                            dst = at[:, 0:C * SW]
                        nc.scalar.activation(
                            dst, src, mybir.ActivationFunctionType.Exp,
                            scale=float(SCALE),
                        )
                        # one strided mask multiply for the whole group
                        u0 = t0 % 8
                        av = at.rearrange("p (g c) -> p g c", c=SW)[:, 0:C, 0:P]
                        nc.vector.tensor_mul(av, av, masks_sb[:, u0:u0 + C, :])
                        # A @ [V | 1] accumulation; the leading
                        # 16(t-8b) at-columns are exact zeros from the
                        # causal mask, so the stream starts past them
                        for ci, t in enumerate(ts):
                            off = 16 * (t - 8 * b)
                            nc.tensor.matmul(
                                ytiles[hh][:, P * b + off:NQ],
                                v_sb[:, t, VW * h:VW * (h + 1)],
                                at[:, SW * ci + off:SW * ci + N],
                                start=(t == 0), stop=(t == NKV - 1),
                                skip_group_check=True,
                            )
                # normalize: y[0:64] * (1 / y[64]) -> yT_sb
                for hh in range(2):
                    h = 2 * hp + hh
                    oc, ro = h // 2, 64 * (h % 2)
                    r = npool.tile([1, NQ], F32, tag="r", name=f"r_{hp}_{hh}")
                    nc.vector.reciprocal(r, ytiles[hh][64:65, :])
                    rbs = npool.tile([64, NQ], F32, tag="rb",
                                     name=f"rb_{hp}_{hh}")
                    nc.gpsimd.partition_broadcast(rbs, r)
                    nc.vector.tensor_tensor(
                        out=yT_sb[ro:ro + 64, oc, :],
                        in0=ytiles[hh][0:64, :], in1=rbs,
                        op=mybir.AluOpType.mult,
                    )

        # ---- output projection (fp16) ------------------------------------
        with (
            tc.tile_pool(name="psum_o", bufs=2, space="PSUM") as pp,
            tc.tile_pool(name="ostage", bufs=3) as opool,
        ):
            for sc in range(NQ // P):
                for og in range(2):
                    ps = pp.tile([P, 384], F32, tag="ps")
                    for ic in range(DMC):
                        nc.tensor.matmul(
                            ps,
                            yT_sb[:, ic, P * sc:P * (sc + 1)],
                            wpT_sb[:, ic, 384 * og:384 * (og + 1)],
                            start=(ic == 0), stop=(ic == DMC - 1),
                        )
                    ost = opool.tile([P, 384], F32, tag="o")
                    nc.vector.tensor_copy(ost, ps)
                    nc.sync.dma_start(
                        out=out[P * sc:P * (sc + 1), 384 * og:384 * (og + 1)],
                        in_=ost,
                    )


def _host_masks(j: int) -> np.ndarray:
    u = np.arange(8)[:, None, None]
    i = np.arange(P)[None, :, None]
    p = np.arange(P)[None, None, :]
    m = (128 * u + i <= 8 * p + j)
    return m.astype(NP16)


def _make_in_maps(xf: np.ndarray, Wq, Wk, Wv, Wp) -> list[dict]:
    """xf: [S, D] fp32; W*: [D, D] fp32 (torch Linear convention y = x W^T)."""
    wts = {}
    for name, W in (("wqT", Wq), ("wkT", Wk), ("wvT", Wv), ("wpT", Wp)):
        wts[name] = np.ascontiguousarray(
            np.asarray(W, np.float32).T.astype(NP16))
    in_maps = []
    for j in range(NCORES):
        in_maps.append({
            "xqT": np.ascontiguousarray(xf[j::NCORES].T.astype(NP16)),
            "xkvT": np.ascontiguousarray(
                xf[SLOT * j:SLOT * (j + 1)].T.astype(NP16)),
            **wts, "masks": _host_masks(j),
        })
    return in_maps


def kernel(x, Wq, Wk, Wv, Wp, **_):
    x = np.asarray(x, dtype=np.float32)
    B = x.shape[0]
    xf = x.reshape(S, D)

    if "nc" not in _CACHE:
        _CACHE["nc"] = _build_program()
    nc = _CACHE["nc"]

    in_maps = _make_in_maps(xf, Wq, Wk, Wv, Wp)
    res = run_bass_kernel_spmd(nc, in_maps, list(range(NCORES)))
    out = np.empty((S, D), np.float32)
    for j in range(NCORES):
        out[j::NCORES] = res.results[j]["out"]
    return out.reshape(B, S, D)


if __name__ == "__main__":
    rng = np.random.default_rng(0)
    x = rng.standard_normal((1, S, D), dtype=np.float32)
    ws = [rng.standard_normal((D, D), dtype=np.float32) / np.sqrt(D)
          for _ in range(4)]
    y = kernel(x, *ws)
    print("ran", y.shape, y.dtype)
